# revision 38
# baseline (speedup 1.0000x reference)
"""Trainium2 Bass kernel for nn_MoEElementFusion (moe_routing).

Strategy (8 NeuronCores, SPMD, two launches with host routing in between):
  Phase 1 (token-data-parallel): each core takes 1/8 of the 8192 (view,token)
  columns, computes in fp16 on the PE (psum fp32):
      h   = x @ proj_w + proj_b
      r   = h @ router_w
      d2X = (-2 keys) . r        and   rr = |r|^2
  Host adds |k|^2 + rr, repairs borderline top-4 rows in fp32, takes stable
  top-4 and softmax gates.

  Slot plan: per expert, selected columns cut into full-512 pieces plus a
  remainder; a small search promotes the largest remainders into full cells
  and groups the rest into short tail slots, minimizing the per-core column
  total (SPMD: every core runs the same compile-time slot-length list; which
  expert fills each slot is pure input data).

  Phase 2 (compiled at runtime once the length list is known): per slot,
  FFN in fp16 (1 cycle/row on the PE):
      out^T = w2^T-mm(gelu(w1^T-mm(h^T) + b1)) + b2

  Perf notes (from NTFF traces):
  - Every large DRAM tensor is host-repacked to partition-major layout so
    DMA descriptor lines are 4-16KB (HWDGE queues are descriptor-rate
    limited at small lines).
  - Weight loads are split across both HWDGE trigger queues (sync=SP,
    scalar=ACT), w2 in two chunks per half so its first m-tiles land early;
    output writes and small/latency-insensitive loads ride gpsimd SWDGE.
  - The PE is pre-warmed with a memset-fed matmul chain into a dedicated
    PSUM bank (drain traced at the end of the program so no engine queue
    head-of-line blocks on it) - the HAM clock gate releases (1.2 -> 2.4
    GHz) while the first inputs stream in.
  - All PSUM output drains run on the otherwise-idle DVE so the ACT engine's
    gelu chain is never interrupted at slot boundaries.
  - Gates are applied on the host during the combine.
"""

import math
import os

import numpy as np

import concourse.bass as bass
import concourse.bacc as bacc
import concourse.mybir as mybir
import concourse.tile as tile
from concourse.bass_utils import run_bass_kernel_spmd

# Problem dims (hardcoded per spec)
V, B, T, D, E, K = 2, 4, 1024, 512, 16, 4
H = 4 * D
N = B * T          # tokens per view
NT = V * N         # total (view, token) columns = 8192
NC = 8             # cores
PC = NT // NC      # phase-1 columns per core = 1024

F16 = mybir.dt.float16
F32 = mybir.dt.float32
AF = mybir.ActivationFunctionType
ALU = mybir.AluOpType

DK = D // 128      # 4 k-tiles over D
HK = H // 128      # 16 k-tiles over H

REPAIR_MARGIN = 0.02
NWARM1 = 17        # phase-1 warmup matmuls
NWARM2 = 22        # phase-2 warmup matmuls
MIN_SLOT = 192     # shorter slots would bottleneck on ACT-engine work

# Filled by kernel() for test harness introspection.
last_stats: dict = {}


def _warmup_start(nc, cpool, wpool, nwarm):
    """Matmul chain on a memset tile accumulating into a dedicated PSUM bank.
    No input-DMA dependency: keeps the PE busy from ~4us so the HAM clock
    gate releases while the first inputs stream in."""
    wz_sb = cpool.tile([128, 512], F16, tag="wz")
    nc.vector.memset(wz_sb[:], 0.0)
    wps = wpool.tile([128, 512], F32, tag="wps")
    for i in range(nwarm):
        nc.tensor.matmul(
            wps[:], wz_sb[:, 0:128], wz_sb[:],
            start=(i == 0), stop=(i == nwarm - 1),
        )
    return wps


def _warmup_end(nc, cpool, wps, wout):
    """Drain the warmup PSUM bank (DVE) and write the dummy output (gpsimd),
    traced at the END of the program so no engine queue blocks on it."""
    wo_sb = cpool.tile([128, 512], F32, tag="wo")
    nc.vector.tensor_scalar(wo_sb[:], wps[:], 0.0, None, ALU.add)
    nc.gpsimd.dma_start(wout[:], wo_sb[:])


def _phase1_nc() -> bass.Bass:
    nc = bacc.Bacc("TRN2", target_bir_lowering=False, num_devices=NC)
    # Partition-major packed inputs (see host-side packing in kernel()).
    xT = nc.dram_tensor("xT", [128, DK * PC], F16, kind="ExternalInput")
    pw = nc.dram_tensor("pw", [128, DK * D], F16, kind="ExternalInput")
    pb = nc.dram_tensor("pb", [128, DK], F32, kind="ExternalInput")
    rw = nc.dram_tensor("rw", [128, DK * D], F16, kind="ExternalInput")
    kT2 = nc.dram_tensor("kT2", [128, DK * E], F16, kind="ExternalInput")
    onc = nc.dram_tensor("onc", [128, 1], F16, kind="ExternalInput")
    hT = nc.dram_tensor("hT", [128, DK * PC], F16, kind="ExternalOutput")
    d2X = nc.dram_tensor("d2X", [E, PC], F32, kind="ExternalOutput")
    rrO = nc.dram_tensor("rrO", [1, PC], F32, kind="ExternalOutput")
    wout = nc.dram_tensor("wout", [128, 512], F32, kind="ExternalOutput")

    NCH = PC // 512  # 512-column compute chunks

    with tile.TileContext(nc) as tc:
        with (
            tc.tile_pool(name="const", bufs=1) as cpool,
            tc.tile_pool(name="act", bufs=1) as apool,
            tc.tile_pool(name="ps", bufs=2, space="PSUM") as pspool,
            tc.tile_pool(name="ps_small", bufs=2, space="PSUM") as psmall,
            tc.tile_pool(name="ps_warm", bufs=1, space="PSUM") as wpool,
        ):
            wps = _warmup_start(nc, cpool, wpool, NWARM1)

            # Inputs: xT per (512-col chunk, k-half), weights per k-half,
            # interleaved across the two HWDGE trigger queues so the first
            # proj chunk's operands land as early as possible.
            # chunk-major xT tile: each chunk DMA is 4KB-contiguous per
            # partition on both sides (128 descriptors, not 512)
            xT_sb = cpool.tile([128, NCH, DK, 512], F16, tag="xT")
            pw_sb = cpool.tile([128, DK, D], F16, tag="pw")
            rw_sb = cpool.tile([128, DK, D], F16, tag="rw")
            nc.sync.dma_start(xT_sb[:, 0], xT[:, 0 : DK * 512])
            nc.scalar.dma_start(pw_sb[:], pw[:])
            nc.sync.dma_start(xT_sb[:, 1], xT[:, DK * 512 : 2 * DK * 512])
            nc.sync.dma_start(rw_sb[:], rw[:])
            pb_sb = cpool.tile([128, DK], F32, tag="pb")
            nc.gpsimd.dma_start(pb_sb[:], pb[:])
            kT2_sb = cpool.tile([128, DK, E], F16, tag="kT2")
            nc.gpsimd.dma_start(kT2_sb[:], kT2[:])
            onc_sb = cpool.tile([128, 1], F16, tag="onc")
            nc.gpsimd.dma_start(onc_sb[:], onc[:])

            hT_sb = apool.tile([128, DK, PC], F16, tag="hT")
            rT_sb = apool.tile([128, DK, PC], F16, tag="rT")
            r2_sb = apool.tile([128, DK, PC], F16, tag="r2")
            rr_sb = apool.tile([1, PC], F32, tag="rr")
            d2_sb = apool.tile([E, PC], F32, tag="d2")

            # h^T = pw^T-mm(x^T) + pb ; r^T = rw^T-mm(h^T)
            for w_sb, chunk_src, dst, bias in (
                (pw_sb, True, hT_sb, pb_sb),
                (rw_sb, False, rT_sb, None),
            ):
                for m in range(DK):
                    for n in range(NCH):
                        ps = pspool.tile([128, 512], F32, tag="ps")
                        for k in range(DK):
                            rhs = (
                                xT_sb[:, n, k, :]
                                if chunk_src
                                else hT_sb[:, k, n * 512 : (n + 1) * 512]
                            )
                            nc.tensor.matmul(
                                ps[:],
                                w_sb[:, k, m * 128 : (m + 1) * 128],
                                rhs,
                                start=(k == 0),
                                stop=(k == DK - 1),
                            )
                        if bias is not None:
                            nc.scalar.activation(
                                dst[:, m, n * 512 : (n + 1) * 512], ps[:],
                                AF.Identity, bias=bias[:, m : m + 1],
                            )
                        else:
                            nc.scalar.activation(
                                dst[:, m, n * 512 : (n + 1) * 512], ps[:], AF.Copy
                            )
                    if bias is not None:
                        # one whole-row store per m: 2KB lines, 128 descriptors
                        nc.sync.dma_start(
                            hT[:, m * PC : (m + 1) * PC], dst[:, m, :]
                        )

            # rr = sum_d r^2 (ones-matmul over partition tiles)
            for kt in range(DK):
                nc.vector.tensor_mul(
                    r2_sb[:, kt, :], rT_sb[:, kt, :], rT_sb[:, kt, :]
                )
            for n in range(NCH):
                ps1 = psmall.tile([1, 512], F32, tag="ps1")
                for k in range(DK):
                    nc.tensor.matmul(
                        ps1[:], onc_sb[:], r2_sb[:, k, n * 512 : (n + 1) * 512],
                        start=(k == 0), stop=(k == DK - 1),
                    )
                nc.scalar.activation(rr_sb[:, n * 512 : (n + 1) * 512], ps1[:], AF.Copy)
            nc.sync.dma_start(rrO[:], rr_sb[:])

            # d2X = (-2 keys) . r   (|k|^2 and rr are added on the host)
            for n in range(NCH):
                psA = psmall.tile([E, 512], F32, tag="psA")
                for k in range(DK):
                    nc.tensor.matmul(
                        psA[:], kT2_sb[:, k, :], rT_sb[:, k, n * 512 : (n + 1) * 512],
                        start=(k == 0), stop=(k == DK - 1),
                    )
                nc.scalar.activation(d2_sb[:, n * 512 : (n + 1) * 512], psA[:], AF.Copy)
            nc.sync.dma_start(d2X[:], d2_sb[:])

            _warmup_end(nc, cpool, wps, wout)
    nc.compile()
    return nc


def _phase2_nc(lens: tuple, loads: tuple) -> bass.Bass:
    S = len(lens)
    offs = [0]
    for L in lens:
        offs.append(offs[-1] + L)
    Ctot = offs[-1]

    nc = bacc.Bacc("TRN2", target_bir_lowering=False, num_devices=NC)
    # Partition-major packed layouts (16KB DMA lines for weights).
    hseg = nc.dram_tensor("hseg", [128, DK * Ctot], F16, kind="ExternalInput")
    w1s = nc.dram_tensor("w1s", [S, 128, DK * H], F16, kind="ExternalInput")
    w2s = nc.dram_tensor("w2s", [S, 128, HK * D], F16, kind="ExternalInput")
    b1s = nc.dram_tensor("b1s", [128, S * HK], F32, kind="ExternalInput")
    b2s = nc.dram_tensor("b2s", [128, S * DK], F32, kind="ExternalInput")
    oseg = nc.dram_tensor("oseg", [128, DK * Ctot], F32, kind="ExternalOutput")
    wout = nc.dram_tensor("wout", [128, 512], F32, kind="ExternalOutput")

    with tile.TileContext(nc) as tc:
        with (
            tc.tile_pool(name="const", bufs=1) as cpool,
            tc.tile_pool(name="w1p", bufs=3) as w1p,
            tc.tile_pool(name="w2p", bufs=3) as w2p,
            tc.tile_pool(name="hp", bufs=3) as hp,
            tc.tile_pool(name="hidp", bufs=4) as hidp,
            tc.tile_pool(name="op", bufs=6) as op,
            tc.tile_pool(name="hid_ps", bufs=3, space="PSUM") as hidps,
            tc.tile_pool(name="out_ps", bufs=1, space="PSUM") as outps,
            tc.tile_pool(name="ps_warm", bufs=1, space="PSUM") as wpool,
        ):
            wps = _warmup_start(nc, cpool, wpool, NWARM2)

            b1_sb = cpool.tile([128, S * HK], F32, tag="b1")
            nc.gpsimd.dma_start(b1_sb[:], b1s[:])
            b2_sb = cpool.tile([128, S * DK], F32, tag="b2")
            nc.gpsimd.dma_start(b2_sb[:], b2s[:])

            w1t = w2t = None
            for s in range(S):
                Lc = lens[s]
                off = offs[s]
                ht = hp.tile([128, DK, 512], F16, tag="h")
                nc.sync.dma_start(
                    ht[:, :, :Lc], hseg[:, off * DK : off * DK + DK * Lc]
                )
                if loads[s]:
                    w1t = w1p.tile([128, DK, H], F16, tag="w1")
                    w2t = w2p.tile([128, HK, D], F16, tag="w2")
                # else: reuse previous slot's w1t/w2t (same expert per core)
                if not loads[s]:
                    pass
                elif s == 0:
                    # Head-latency path: w1 per (k, m-half) so the first
                    # m-tiles land ~3us earlier; queues are empty here so the
                    # finer descriptor lines don't throttle anything.
                    HH = H // 2
                    for hh in range(2):
                        for k, eng in ((0, nc.sync), (1, nc.sync),
                                       (2, nc.scalar), (3, nc.scalar)):
                            eng.dma_start(
                                w1t[:, k, hh * HH : (hh + 1) * HH],
                                w1s[s, :, k * H + hh * HH : k * H + (hh + 1) * HH],
                            )
                        for q, eng in ((2 * hh, nc.sync), (2 * hh + 1, nc.scalar)):
                            eng.dma_start(
                                w2t[:, 4 * q : 4 * q + 4, :],
                                w2s[s, :, 4 * q * D : (4 * q + 4) * D],
                            )
                else:
                    # Steady state: k-half / m-half loads with 8KB lines keep
                    # the HWDGE descriptor rate low.
                    nc.sync.dma_start(w1t[:, 0:2, :], w1s[s, :, : 2 * H])
                    nc.sync.dma_start(w1t[:, 2:4, :], w1s[s, :, 2 * H :])
                    nc.sync.dma_start(w2t[:, 0:8, :], w2s[s, :, : 8 * D])
                    nc.sync.dma_start(w2t[:, 8:16, :], w2s[s, :, 8 * D :])
                opsums = [
                    outps.tile([128, 512], F32, tag=f"opsum{mo}",
                               name=f"opsum_{s}_{mo}")
                    for mo in range(DK)
                ]
                for m in range(HK):
                    hps = hidps.tile([128, 512], F32, tag="hps")
                    for k in range(DK):
                        nc.tensor.matmul(
                            hps[:, :Lc],
                            w1t[:, k, m * 128 : (m + 1) * 128],
                            ht[:, k, :Lc],
                            start=(k == 0),
                            stop=(k == DK - 1),
                        )
                    hidt = hidp.tile([128, 512], F16, tag="hid")
                    nc.scalar.activation(
                        hidt[:, :Lc], hps[:, :Lc], AF.Gelu,
                        bias=b1_sb[:, s * HK + m : s * HK + m + 1],
                    )
                    for mo in range(DK):
                        nc.tensor.matmul(
                            opsums[mo][:, :Lc],
                            w2t[:, m, mo * 128 : (mo + 1) * 128],
                            hidt[:, :Lc],
                            start=(m == 0),
                            stop=(m == HK - 1),
                        )
                # All drains on DVE; separate single-bank opsum tiles release
                # one by one so the next slot's w2 accumulation starts early.
                for mo in range(DK):
                    ot = op.tile([128, 512], F32, tag="o")
                    bcol = b2_sb[:, s * DK + mo : s * DK + mo + 1]
                    nc.vector.tensor_scalar(
                        ot[:, :Lc], opsums[mo][:, :Lc], bcol, None, ALU.add
                    )
                    if s == S - 1:  # final slot: HWDGE only (snappier tail)
                        eng = nc.sync
                    else:
                        eng = (nc.gpsimd, nc.gpsimd, nc.sync, nc.gpsimd)[mo]
                    eng.dma_start(
                        oseg[:, off * DK + mo * Lc : off * DK + (mo + 1) * Lc],
                        ot[:, :Lc],
                    )

            _warmup_end(nc, cpool, wps, wout)
    nc.compile()
    return nc


def _run(nc, in_maps, label):
    trace = os.environ.get("KTRACE") == "1"
    res = run_bass_kernel_spmd(
        nc, in_maps, core_ids=list(range(NC)), trace=trace
    )
    if trace:
        last_stats[label] = {
            "exec_time_ns": res.exec_time_ns,
            "mean_exec_time_ns": res.mean_exec_time_ns,
            "trace": res.instructions_and_trace[1]
            if res.instructions_and_trace
            else None,
        }
    return res.results


def _round16(x: int) -> int:
    return (x + 7) & ~7


def _pack_pmajor(a: np.ndarray, kt: int) -> np.ndarray:
    """[kt*128, F] row-major -> [128, kt*F] partition-major."""
    f = a.shape[1]
    return np.ascontiguousarray(
        a.reshape(kt, 128, f).transpose(1, 0, 2).reshape(128, kt * f)
    )


def _plan_slots(topi, gates):
    """Cut each expert's selected columns into pieces and pick compile-time
    slot lengths minimizing the per-core column total (SPMD-uniform)."""
    per_e = []  # (token_ids, gate_vals)
    for e in range(E):
        sel_tok, sel_k = np.nonzero(topi == e)
        per_e.append((sel_tok, gates[sel_tok, sel_k]))

    full_pieces = []  # (e, start, 512)
    rem_pieces = []   # (e, start, len)
    for e in range(E):
        n = per_e[e][0].size
        f, r = divmod(n, 512)
        for i in range(f):
            full_pieces.append((e, i * 512, 512))
        if r:
            rem_pieces.append((e, f * 512, r))
    rem_pieces.sort(key=lambda p: -p[2])

    Fn = len(full_pieces)
    best = None
    for p in range(len(rem_pieces) + 1):
        n512 = Fn + p
        S512 = max(1, math.ceil(n512 / NC))
        rest = rem_pieces[p:]
        tlens = [
            max(MIN_SLOT, _round16(rest[i * NC][2]))
            for i in range(math.ceil(len(rest) / NC))
        ]
        cost = 512 * S512 + sum(tlens)
        if best is None or cost < best[0]:
            best = (cost, p, S512, tlens)
    _, p, S512, tlens = best

    full_cells = full_pieces + rem_pieces[:p]
    full_cells += [None] * (S512 * NC - len(full_cells))
    rest = rem_pieces[p:]

    # Same-expert cell PAIRS per core let the second slot of a pair skip its
    # 4MiB weight load (compile-time flag, SPMD-uniform): group full cells by
    # expert, emit floor(pairs/8) slot-pairs, the rest as unpaired slots.
    by_e = {}
    for cell in full_cells:
        if cell is not None:
            by_e.setdefault(cell[0], []).append(cell)
    pairs = []
    singles = []
    for e, cells in by_e.items():
        n2 = len(cells) // 2
        for i in range(n2):
            pairs.append((cells[2 * i], cells[2 * i + 1]))
        if len(cells) % 2:
            singles.append(cells[-1])
    n_sp = len(pairs) // NC
    slots = []   # (length, [8 cells])
    loads = []   # True if slot s loads fresh weights
    for j in range(n_sp):
        grp = pairs[j * NC : (j + 1) * NC]
        slots.append((512, [g[0] for g in grp]))
        loads.append(True)
        slots.append((512, [g[1] for g in grp]))
        loads.append(False)
    for pr in pairs[n_sp * NC :]:
        singles.extend(pr)
    singles += [None] * ((-len(singles)) % NC)
    for i in range(len(singles) // NC):
        slots.append((512, singles[i * NC : (i + 1) * NC]))
        loads.append(True)
    for i, tl in enumerate(tlens):
        cells = rest[i * NC : (i + 1) * NC]
        cells += [None] * (NC - len(cells))
        slots.append((tl, cells))  # short slots last: smaller kernel tail
        loads.append(True)
    return per_e, slots, loads


def kernel(view0, view1, proj_w, proj_b, router_w, expert_keys, w1, b1, w2, b2):
    view0 = np.ascontiguousarray(view0, dtype=np.float32)
    view1 = np.ascontiguousarray(view1, dtype=np.float32)
    proj_w = np.asarray(proj_w, dtype=np.float32)
    proj_b = np.asarray(proj_b, dtype=np.float32)
    router_w = np.asarray(router_w, dtype=np.float32)
    keys = np.asarray(expert_keys, dtype=np.float32)
    w1 = np.asarray(w1, dtype=np.float32)
    b1 = np.asarray(b1, dtype=np.float32)
    w2 = np.asarray(w2, dtype=np.float32)
    b2 = np.asarray(b2, dtype=np.float32)

    # ---- Phase 1: h, cross-term d2X and rr on device ----
    xT_full = np.concatenate(
        [view0.reshape(N, D).T, view1.reshape(N, D).T], axis=1
    ).astype(np.float16)  # [D, NT], column t = view*N + (b*T + tt)

    kT2 = _pack_pmajor(np.ascontiguousarray(-2.0 * keys.T).astype(np.float16), DK)
    kk = (keys * keys).sum(axis=1, dtype=np.float32)  # [E]
    onc = np.ones((128, 1), np.float16)

    in_maps1 = []
    for c in range(NC):
        v = (c * PC) // N  # cores 0-3 -> view 0, 4-7 -> view 1
        xc = xT_full[:, c * PC : (c + 1) * PC]  # [D, PC]
        # chunk-contiguous packing: [128, (n512, k, c)]
        xr = np.ascontiguousarray(
            xc.reshape(DK, 128, PC // 512, 512)
            .transpose(1, 2, 0, 3)
            .reshape(128, DK * PC)
        )
        in_maps1.append(
            {
                "xT": xr,
                "pw": _pack_pmajor(proj_w[v].astype(np.float16), DK),
                "pb": np.ascontiguousarray(proj_b[v].reshape(DK, 128).T),
                "rw": _pack_pmajor(router_w[v].astype(np.float16), DK),
                "kT2": kT2,
                "onc": onc,
            }
        )
    res1 = _run(_phase1_nc(), in_maps1, "phase1")

    # hT output layout [128, (m, col)] -> [D, PC] per core
    hT_d = np.concatenate(
        [
            r["hT"].reshape(128, DK, PC).transpose(1, 0, 2).reshape(D, PC)
            for r in res1
        ],
        axis=1,
    )  # [D, NT] f16
    d2 = np.concatenate([r["d2X"] for r in res1], axis=1).T      # [NT, E] f32
    rr = np.concatenate([r["rrO"] for r in res1], axis=1).T      # [NT, 1] f32
    d2 += rr
    d2 += kk[None, :]

    # ---- Host repair: recompute borderline tokens exactly in fp32 ----
    logits0 = -np.sqrt(np.maximum(d2, 0.0), dtype=np.float32)
    part = np.partition(logits0, E - K - 1, axis=1)
    gap45 = part[:, E - K] - part[:, E - K - 1]  # 4th minus 5th logit
    risk = np.nonzero(gap45 < REPAIR_MARGIN)[0]
    last_stats["n_repaired"] = int(risk.size)
    if risk.size:
        x_all = np.concatenate(
            [view0.reshape(N, D), view1.reshape(N, D)], axis=0
        )
        vsel = (risk >= N).astype(np.int64)
        for v in (0, 1):
            rt = risk[vsel == v]
            if rt.size == 0:
                continue
            hx = x_all[rt] @ proj_w[v] + proj_b[v]
            rx = hx @ router_w[v]
            d2[rt] = (
                (rx * rx).sum(axis=1, keepdims=True)
                - 2.0 * (rx @ keys.T)
                + kk
            )

    # ---- Host routing: logits, top-4, softmax gates (fp32) ----
    logits = -np.sqrt(np.maximum(d2, 0.0), dtype=np.float32)
    topi = np.argsort(-logits, axis=1, kind="stable")[:, :K]   # [NT, K]
    topv = np.take_along_axis(logits, topi, axis=1)
    ex = np.exp(topv - topv[:, :1], dtype=np.float32)
    gates = ex / ex.sum(axis=1, keepdims=True, dtype=np.float32)

    # ---- Slot plan ----
    per_e, slots, loads = _plan_slots(topi, gates)
    lens = tuple(sl[0] for sl in slots)
    S = len(lens)
    offs = np.concatenate([[0], np.cumsum(lens)]).astype(np.int64)
    Ctot = int(offs[-1])

    # ---- Phase 2 inputs (partition-major packed) ----
    w1r = np.stack([_pack_pmajor(w1[e].astype(np.float16), DK) for e in range(E)])
    w2r = np.stack([_pack_pmajor(w2[e].astype(np.float16), HK) for e in range(E)])
    hT_p = np.ascontiguousarray(
        hT_d.reshape(DK, 128, NT).transpose(1, 0, 2)
    )  # [128, DK, NT]
    in_maps2 = []
    core_cells = []  # per core: list over slots of (e, toks, gvals) or None
    for c in range(NC):
        hseg = np.zeros((128, DK * Ctot), np.float16)
        w1c = np.zeros((S, 128, DK * H), np.float16)
        w2c = np.zeros((S, 128, HK * D), np.float16)
        b1c = np.zeros((128, S * HK), np.float32)
        b2c = np.zeros((128, S * DK), np.float32)
        cells = []
        for s, (Lc, cell8) in enumerate(slots):
            cell = cell8[c]
            if cell is None:
                cells.append(None)
                continue
            e, start, n = cell
            toks = per_e[e][0][start : start + n]
            gv = per_e[e][1][start : start + n]
            cells.append((e, toks, gv))
            blk = hT_p[:, :, toks]  # [128, DK, n]
            o0 = int(offs[s]) * DK
            hs = hseg[:, o0 : o0 + DK * Lc].reshape(128, DK, Lc)  # strided view
            hs[:, :, :n] = blk
            w1c[s] = w1r[e]
            w2c[s] = w2r[e]
            b1c[:, s * HK : (s + 1) * HK] = b1[e].reshape(HK, 128).T
            b2c[:, s * DK : (s + 1) * DK] = b2[e].reshape(DK, 128).T
        core_cells.append(cells)
        in_maps2.append(
            {"hseg": hseg, "w1s": w1c, "w2s": w2c, "b1s": b1c, "b2s": b2c}
        )
    last_stats["S"] = S
    last_stats["n_slots_real"] = sum(
        1 for cells in core_cells for cl in cells if cl is not None
    )
    last_stats["cols_per_core"] = Ctot
    last_stats["n_wloads"] = sum(loads)
    res2 = _run(_phase2_nc(lens, tuple(loads)), in_maps2, "phase2")

    # ---- Combine (gates applied here) ----
    fusedT = np.zeros((D, NT), np.float32)
    for c in range(NC):
        o = res2[c]["oseg"]  # [128, DK*Ctot]
        for s in range(S):
            cell = core_cells[c][s]
            if cell is None:
                continue
            e, toks, gv = cell
            n = toks.size
            Lc = lens[s]
            o0 = int(offs[s]) * DK
            blk = o[:, o0 : o0 + DK * Lc].reshape(128, DK, Lc)[:, :, :n]
            fusedT[:, toks] += (
                blk.transpose(1, 0, 2).reshape(D, n) * gv[None, :]
            )
    fused = (fusedT[:, :N] + fusedT[:, N:]).T  # [N, D]
    return np.ascontiguousarray(fused.reshape(B, T, D), dtype=np.float32)


# revision 39
# speedup vs baseline: 1.1949x; 1.1949x over previous
"""Trainium2 Bass kernel for nn_MoEElementFusion (moe_routing).

Strategy (8 NeuronCores, SPMD, two launches with host routing in between):
  Phase 1 (token-data-parallel): each core takes 1/8 of the 8192 (view,token)
  columns, computes in fp16 on the PE (psum fp32):
      h   = x @ proj_w + proj_b
      r   = h @ router_w
      d2X = (-2 keys) . r        and   rr = |r|^2
  Host adds |k|^2 + rr, repairs borderline top-4 rows in fp32, takes stable
  top-4 and softmax gates.

  Slot plan: per expert, selected columns cut into full-512 pieces plus a
  remainder; a small search promotes the largest remainders into full cells
  and groups the rest into short tail slots, minimizing the per-core column
  total (SPMD: every core runs the same compile-time slot-length list; which
  expert fills each slot is pure input data).

  Phase 2 (compiled at runtime once the length list is known): per slot,
  FFN in fp16 (1 cycle/row on the PE):
      out^T = w2^T-mm(gelu(w1^T-mm(h^T) + b1)) + b2

  Perf notes (from NTFF traces):
  - Every large DRAM tensor is host-repacked to partition-major layout so
    DMA descriptor lines are 4-16KB (HWDGE queues are descriptor-rate
    limited at small lines).
  - Weight loads are split across both HWDGE trigger queues (sync=SP,
    scalar=ACT), w2 in two chunks per half so its first m-tiles land early;
    output writes and small/latency-insensitive loads ride gpsimd SWDGE.
  - The PE is pre-warmed with a memset-fed matmul chain into a dedicated
    PSUM bank (drain traced at the end of the program so no engine queue
    head-of-line blocks on it) - the HAM clock gate releases (1.2 -> 2.4
    GHz) while the first inputs stream in.
  - All PSUM output drains run on the otherwise-idle DVE so the ACT engine's
    gelu chain is never interrupted at slot boundaries.
  - Gates are applied on the host during the combine.
"""

import math
import os

import numpy as np

import concourse.bass as bass
import concourse.bacc as bacc
import concourse.mybir as mybir
import concourse.tile as tile
from concourse.bass_utils import run_bass_kernel_spmd

# Problem dims (hardcoded per spec)
V, B, T, D, E, K = 2, 4, 1024, 512, 16, 4
H = 4 * D
N = B * T          # tokens per view
NT = V * N         # total (view, token) columns = 8192
NC = 8             # cores
PC = NT // NC      # phase-1 columns per core = 1024

F16 = mybir.dt.float16
F32 = mybir.dt.float32
AF = mybir.ActivationFunctionType
ALU = mybir.AluOpType

DK = D // 128      # 4 k-tiles over D
HK = H // 128      # 16 k-tiles over H

REPAIR_MARGIN = 0.02
NWARM1 = 17        # phase-1 warmup matmuls
NWARM2 = 22        # phase-2 warmup matmuls
MIN_SLOT = 192     # shorter slots would bottleneck on ACT-engine work

# Filled by kernel() for test harness introspection.
last_stats: dict = {}


def _warmup_start(nc, cpool, wpool, nwarm):
    """Matmul chain on a memset tile accumulating into a dedicated PSUM bank.
    No input-DMA dependency: keeps the PE busy from ~4us so the HAM clock
    gate releases while the first inputs stream in."""
    wz_sb = cpool.tile([128, 512], F16, tag="wz")
    nc.vector.memset(wz_sb[:], 0.0)
    wps = wpool.tile([128, 512], F32, tag="wps")
    for i in range(nwarm):
        nc.tensor.matmul(
            wps[:], wz_sb[:, 0:128], wz_sb[:],
            start=(i == 0), stop=(i == nwarm - 1),
        )
    return wps


def _warmup_end(nc, cpool, wps, wout):
    """Drain the warmup PSUM bank (DVE) and write the dummy output (gpsimd),
    traced at the END of the program so no engine queue blocks on it."""
    wo_sb = cpool.tile([128, 512], F32, tag="wo")
    nc.vector.tensor_scalar(wo_sb[:], wps[:], 0.0, None, ALU.add)
    nc.gpsimd.dma_start(wout[:], wo_sb[:])


def _phase1_nc() -> bass.Bass:
    nc = bacc.Bacc("TRN2", target_bir_lowering=False, num_devices=NC)
    # Partition-major packed inputs (see host-side packing in kernel()).
    xT = nc.dram_tensor("xT", [128, DK * PC], F16, kind="ExternalInput")
    pw = nc.dram_tensor("pw", [128, DK * D], F16, kind="ExternalInput")
    pb = nc.dram_tensor("pb", [128, DK], F32, kind="ExternalInput")
    rw = nc.dram_tensor("rw", [128, DK * D], F16, kind="ExternalInput")
    kT2 = nc.dram_tensor("kT2", [128, DK * E], F16, kind="ExternalInput")
    onc = nc.dram_tensor("onc", [128, 1], F16, kind="ExternalInput")
    hT = nc.dram_tensor("hT", [128, DK * PC], F16, kind="ExternalOutput")
    d2X = nc.dram_tensor("d2X", [E, PC], F32, kind="ExternalOutput")
    rrO = nc.dram_tensor("rrO", [1, PC], F32, kind="ExternalOutput")
    wout = nc.dram_tensor("wout", [128, 512], F32, kind="ExternalOutput")

    NCH = PC // 512  # 512-column compute chunks

    with tile.TileContext(nc) as tc:
        with (
            tc.tile_pool(name="const", bufs=1) as cpool,
            tc.tile_pool(name="act", bufs=1) as apool,
            tc.tile_pool(name="ps", bufs=2, space="PSUM") as pspool,
            tc.tile_pool(name="ps_small", bufs=2, space="PSUM") as psmall,
            tc.tile_pool(name="ps_warm", bufs=1, space="PSUM") as wpool,
        ):
            wps = _warmup_start(nc, cpool, wpool, NWARM1)

            # Inputs: xT per (512-col chunk, k-half), weights per k-half,
            # interleaved across the two HWDGE trigger queues so the first
            # proj chunk's operands land as early as possible.
            # chunk-major xT tile: each chunk DMA is 4KB-contiguous per
            # partition on both sides (128 descriptors, not 512)
            xT_sb = cpool.tile([128, NCH, DK, 512], F16, tag="xT")
            pw_sb = cpool.tile([128, DK, D], F16, tag="pw")
            rw_sb = cpool.tile([128, DK, D], F16, tag="rw")
            nc.sync.dma_start(xT_sb[:, 0], xT[:, 0 : DK * 512])
            nc.scalar.dma_start(pw_sb[:], pw[:])
            nc.sync.dma_start(xT_sb[:, 1], xT[:, DK * 512 : 2 * DK * 512])
            nc.sync.dma_start(rw_sb[:], rw[:])
            pb_sb = cpool.tile([128, DK], F32, tag="pb")
            nc.gpsimd.dma_start(pb_sb[:], pb[:])
            kT2_sb = cpool.tile([128, DK, E], F16, tag="kT2")
            nc.gpsimd.dma_start(kT2_sb[:], kT2[:])
            onc_sb = cpool.tile([128, 1], F16, tag="onc")
            nc.gpsimd.dma_start(onc_sb[:], onc[:])

            hT_sb = apool.tile([128, DK, PC], F16, tag="hT")
            rT_sb = apool.tile([128, DK, PC], F16, tag="rT")
            r2_sb = apool.tile([128, DK, PC], F16, tag="r2")
            rr_sb = apool.tile([1, PC], F32, tag="rr")
            d2_sb = apool.tile([E, PC], F32, tag="d2")

            # h^T = pw^T-mm(x^T) + pb ; r^T = rw^T-mm(h^T)
            for w_sb, chunk_src, dst, bias in (
                (pw_sb, True, hT_sb, pb_sb),
                (rw_sb, False, rT_sb, None),
            ):
                for m in range(DK):
                    for n in range(NCH):
                        ps = pspool.tile([128, 512], F32, tag="ps")
                        for k in range(DK):
                            rhs = (
                                xT_sb[:, n, k, :]
                                if chunk_src
                                else hT_sb[:, k, n * 512 : (n + 1) * 512]
                            )
                            nc.tensor.matmul(
                                ps[:],
                                w_sb[:, k, m * 128 : (m + 1) * 128],
                                rhs,
                                start=(k == 0),
                                stop=(k == DK - 1),
                            )
                        if bias is not None:
                            nc.scalar.activation(
                                dst[:, m, n * 512 : (n + 1) * 512], ps[:],
                                AF.Identity, bias=bias[:, m : m + 1],
                            )
                        else:
                            nc.scalar.activation(
                                dst[:, m, n * 512 : (n + 1) * 512], ps[:], AF.Copy
                            )
                    if bias is not None:
                        # one whole-row store per m: 2KB lines, 128 descriptors
                        eng = nc.sync if m % 2 else nc.scalar
                        eng.dma_start(
                            hT[:, m * PC : (m + 1) * PC], dst[:, m, :]
                        )

            # rr = sum_d r^2 (ones-matmul over partition tiles)
            for kt in range(DK):
                nc.vector.tensor_mul(
                    r2_sb[:, kt, :], rT_sb[:, kt, :], rT_sb[:, kt, :]
                )
            for n in range(NCH):
                ps1 = psmall.tile([1, 512], F32, tag="ps1")
                for k in range(DK):
                    nc.tensor.matmul(
                        ps1[:], onc_sb[:], r2_sb[:, k, n * 512 : (n + 1) * 512],
                        start=(k == 0), stop=(k == DK - 1),
                    )
                nc.scalar.activation(rr_sb[:, n * 512 : (n + 1) * 512], ps1[:], AF.Copy)
            nc.scalar.dma_start(rrO[:], rr_sb[:])

            # d2X = (-2 keys) . r   (|k|^2 and rr are added on the host)
            for n in range(NCH):
                psA = psmall.tile([E, 512], F32, tag="psA")
                for k in range(DK):
                    nc.tensor.matmul(
                        psA[:], kT2_sb[:, k, :], rT_sb[:, k, n * 512 : (n + 1) * 512],
                        start=(k == 0), stop=(k == DK - 1),
                    )
                nc.scalar.activation(d2_sb[:, n * 512 : (n + 1) * 512], psA[:], AF.Copy)
            nc.sync.dma_start(d2X[:], d2_sb[:])

            _warmup_end(nc, cpool, wps, wout)
    nc.compile()
    return nc


def _phase2_nc(lens: tuple, loads: tuple) -> bass.Bass:
    S = len(lens)
    offs = [0]
    for L in lens:
        offs.append(offs[-1] + L)
    Ctot = offs[-1]

    nc = bacc.Bacc("TRN2", target_bir_lowering=False, num_devices=NC)
    # Partition-major packed layouts (16KB DMA lines for weights).
    hseg = nc.dram_tensor("hseg", [128, DK * Ctot], F16, kind="ExternalInput")
    w1s = nc.dram_tensor("w1s", [S, 128, DK * H], F16, kind="ExternalInput")
    w2s = nc.dram_tensor("w2s", [S, 128, HK * D], F16, kind="ExternalInput")
    b1s = nc.dram_tensor("b1s", [128, S * HK], F32, kind="ExternalInput")
    b2s = nc.dram_tensor("b2s", [128, S * DK], F32, kind="ExternalInput")
    oseg = nc.dram_tensor("oseg", [128, DK * Ctot], F32, kind="ExternalOutput")
    wout = nc.dram_tensor("wout", [128, 512], F32, kind="ExternalOutput")

    with tile.TileContext(nc) as tc:
        with (
            tc.tile_pool(name="const", bufs=1) as cpool,
            tc.tile_pool(name="w1p", bufs=3) as w1p,
            tc.tile_pool(name="w2p", bufs=3) as w2p,
            tc.tile_pool(name="hp", bufs=3) as hp,
            tc.tile_pool(name="hidp", bufs=4) as hidp,
            tc.tile_pool(name="op", bufs=6) as op,
            tc.tile_pool(name="hid_ps", bufs=3, space="PSUM") as hidps,
            tc.tile_pool(name="out_ps", bufs=1, space="PSUM") as outps,
            tc.tile_pool(name="ps_warm", bufs=1, space="PSUM") as wpool,
        ):
            wps = _warmup_start(nc, cpool, wpool, NWARM2)

            b1_sb = cpool.tile([128, S * HK], F32, tag="b1")
            nc.gpsimd.dma_start(b1_sb[:], b1s[:])
            b2_sb = cpool.tile([128, S * DK], F32, tag="b2")
            nc.gpsimd.dma_start(b2_sb[:], b2s[:])

            w1t = w2t = None
            for s in range(S):
                Lc = lens[s]
                off = offs[s]
                ht = hp.tile([128, DK, 512], F16, tag="h")
                nc.sync.dma_start(
                    ht[:, :, :Lc], hseg[:, off * DK : off * DK + DK * Lc]
                )
                if loads[s]:
                    w1t = w1p.tile([128, DK, H], F16, tag="w1")
                    w2t = w2p.tile([128, HK, D], F16, tag="w2")
                # else: reuse previous slot's w1t/w2t (same expert per core)
                if not loads[s]:
                    pass
                elif s == 0:
                    # Head-latency path: w1 per (k, m-half) so the first
                    # m-tiles land ~3us earlier; queues are empty here so the
                    # finer descriptor lines don't throttle anything.
                    HH = H // 2
                    for hh in range(2):
                        for k, eng in ((0, nc.sync), (1, nc.sync),
                                       (2, nc.scalar), (3, nc.scalar)):
                            eng.dma_start(
                                w1t[:, k, hh * HH : (hh + 1) * HH],
                                w1s[s, :, k * H + hh * HH : k * H + (hh + 1) * HH],
                            )
                        for q, eng in ((2 * hh, nc.sync), (2 * hh + 1, nc.scalar)):
                            eng.dma_start(
                                w2t[:, 4 * q : 4 * q + 4, :],
                                w2s[s, :, 4 * q * D : (4 * q + 4) * D],
                            )
                else:
                    # Steady state: k-half / m-half loads with 8KB lines keep
                    # the HWDGE descriptor rate low.
                    nc.sync.dma_start(w1t[:, 0:2, :], w1s[s, :, : 2 * H])
                    nc.scalar.dma_start(w1t[:, 2:4, :], w1s[s, :, 2 * H :])
                    nc.sync.dma_start(w2t[:, 0:8, :], w2s[s, :, : 8 * D])
                    nc.scalar.dma_start(w2t[:, 8:16, :], w2s[s, :, 8 * D :])
                opsums = [
                    outps.tile([128, 512], F32, tag=f"opsum{mo}",
                               name=f"opsum_{s}_{mo}")
                    for mo in range(DK)
                ]
                for m in range(HK):
                    hps = hidps.tile([128, 512], F32, tag="hps")
                    for k in range(DK):
                        nc.tensor.matmul(
                            hps[:, :Lc],
                            w1t[:, k, m * 128 : (m + 1) * 128],
                            ht[:, k, :Lc],
                            start=(k == 0),
                            stop=(k == DK - 1),
                        )
                    hidt = hidp.tile([128, 512], F16, tag="hid")
                    nc.scalar.activation(
                        hidt[:, :Lc], hps[:, :Lc], AF.Gelu,
                        bias=b1_sb[:, s * HK + m : s * HK + m + 1],
                    )
                    for mo in range(DK):
                        nc.tensor.matmul(
                            opsums[mo][:, :Lc],
                            w2t[:, m, mo * 128 : (mo + 1) * 128],
                            hidt[:, :Lc],
                            start=(m == 0),
                            stop=(m == HK - 1),
                        )
                # All drains on DVE; separate single-bank opsum tiles release
                # one by one so the next slot's w2 accumulation starts early.
                for mo in range(DK):
                    ot = op.tile([128, 512], F32, tag="o")
                    bcol = b2_sb[:, s * DK + mo : s * DK + mo + 1]
                    nc.vector.tensor_scalar(
                        ot[:, :Lc], opsums[mo][:, :Lc], bcol, None, ALU.add
                    )
                    if s == S - 1:  # final slot: HWDGE only (snappier tail)
                        eng = (nc.sync, nc.scalar, nc.sync, nc.scalar)[mo]
                    else:
                        eng = (nc.gpsimd, nc.gpsimd, nc.sync, nc.scalar)[mo]
                    eng.dma_start(
                        oseg[:, off * DK + mo * Lc : off * DK + (mo + 1) * Lc],
                        ot[:, :Lc],
                    )

            _warmup_end(nc, cpool, wps, wout)
    nc.compile()
    return nc


def _run(nc, in_maps, label):
    trace = os.environ.get("KTRACE") == "1"
    res = run_bass_kernel_spmd(
        nc, in_maps, core_ids=list(range(NC)), trace=trace
    )
    if trace:
        last_stats[label] = {
            "exec_time_ns": res.exec_time_ns,
            "mean_exec_time_ns": res.mean_exec_time_ns,
            "trace": res.instructions_and_trace[1]
            if res.instructions_and_trace
            else None,
        }
    return res.results


def _round16(x: int) -> int:
    return (x + 7) & ~7


def _pack_pmajor(a: np.ndarray, kt: int) -> np.ndarray:
    """[kt*128, F] row-major -> [128, kt*F] partition-major."""
    f = a.shape[1]
    return np.ascontiguousarray(
        a.reshape(kt, 128, f).transpose(1, 0, 2).reshape(128, kt * f)
    )


def _plan_slots(topi, gates):
    """Cut each expert's selected columns into pieces and pick compile-time
    slot lengths minimizing the per-core column total (SPMD-uniform)."""
    per_e = []  # (token_ids, gate_vals)
    for e in range(E):
        sel_tok, sel_k = np.nonzero(topi == e)
        per_e.append((sel_tok, gates[sel_tok, sel_k]))

    full_pieces = []  # (e, start, 512)
    rem_pieces = []   # (e, start, len)
    for e in range(E):
        n = per_e[e][0].size
        f, r = divmod(n, 512)
        for i in range(f):
            full_pieces.append((e, i * 512, 512))
        if r:
            rem_pieces.append((e, f * 512, r))
    rem_pieces.sort(key=lambda p: -p[2])

    Fn = len(full_pieces)
    best = None
    for p in range(len(rem_pieces) + 1):
        n512 = Fn + p
        S512 = max(1, math.ceil(n512 / NC))
        rest = rem_pieces[p:]
        tlens = [
            max(MIN_SLOT, _round16(rest[i * NC][2]))
            for i in range(math.ceil(len(rest) / NC))
        ]
        cost = 512 * S512 + sum(tlens)
        if best is None or cost < best[0]:
            best = (cost, p, S512, tlens)
    _, p, S512, tlens = best

    full_cells = full_pieces + rem_pieces[:p]
    full_cells += [None] * (S512 * NC - len(full_cells))
    rest = rem_pieces[p:]

    # Same-expert cell PAIRS per core let the second slot of a pair skip its
    # 4MiB weight load (compile-time flag, SPMD-uniform): group full cells by
    # expert, emit floor(pairs/8) slot-pairs, the rest as unpaired slots.
    by_e = {}
    for cell in full_cells:
        if cell is not None:
            by_e.setdefault(cell[0], []).append(cell)
    pairs = []
    singles = []
    for e, cells in by_e.items():
        n2 = len(cells) // 2
        for i in range(n2):
            pairs.append((cells[2 * i], cells[2 * i + 1]))
        if len(cells) % 2:
            singles.append(cells[-1])
    n_sp = len(pairs) // NC
    slots = []   # (length, [8 cells])
    loads = []   # True if slot s loads fresh weights
    for j in range(n_sp):
        grp = pairs[j * NC : (j + 1) * NC]
        slots.append((512, [g[0] for g in grp]))
        loads.append(True)
        slots.append((512, [g[1] for g in grp]))
        loads.append(False)
    for pr in pairs[n_sp * NC :]:
        singles.extend(pr)
    singles += [None] * ((-len(singles)) % NC)
    for i in range(len(singles) // NC):
        slots.append((512, singles[i * NC : (i + 1) * NC]))
        loads.append(True)
    for i, tl in enumerate(tlens):
        cells = rest[i * NC : (i + 1) * NC]
        cells += [None] * (NC - len(cells))
        slots.append((tl, cells))  # short slots last: smaller kernel tail
        loads.append(True)
    return per_e, slots, loads


def kernel(view0, view1, proj_w, proj_b, router_w, expert_keys, w1, b1, w2, b2):
    view0 = np.ascontiguousarray(view0, dtype=np.float32)
    view1 = np.ascontiguousarray(view1, dtype=np.float32)
    proj_w = np.asarray(proj_w, dtype=np.float32)
    proj_b = np.asarray(proj_b, dtype=np.float32)
    router_w = np.asarray(router_w, dtype=np.float32)
    keys = np.asarray(expert_keys, dtype=np.float32)
    w1 = np.asarray(w1, dtype=np.float32)
    b1 = np.asarray(b1, dtype=np.float32)
    w2 = np.asarray(w2, dtype=np.float32)
    b2 = np.asarray(b2, dtype=np.float32)

    # ---- Phase 1: h, cross-term d2X and rr on device ----
    xT_full = np.concatenate(
        [view0.reshape(N, D).T, view1.reshape(N, D).T], axis=1
    ).astype(np.float16)  # [D, NT], column t = view*N + (b*T + tt)

    kT2 = _pack_pmajor(np.ascontiguousarray(-2.0 * keys.T).astype(np.float16), DK)
    kk = (keys * keys).sum(axis=1, dtype=np.float32)  # [E]
    onc = np.ones((128, 1), np.float16)

    in_maps1 = []
    for c in range(NC):
        v = (c * PC) // N  # cores 0-3 -> view 0, 4-7 -> view 1
        xc = xT_full[:, c * PC : (c + 1) * PC]  # [D, PC]
        # chunk-contiguous packing: [128, (n512, k, c)]
        xr = np.ascontiguousarray(
            xc.reshape(DK, 128, PC // 512, 512)
            .transpose(1, 2, 0, 3)
            .reshape(128, DK * PC)
        )
        in_maps1.append(
            {
                "xT": xr,
                "pw": _pack_pmajor(proj_w[v].astype(np.float16), DK),
                "pb": np.ascontiguousarray(proj_b[v].reshape(DK, 128).T),
                "rw": _pack_pmajor(router_w[v].astype(np.float16), DK),
                "kT2": kT2,
                "onc": onc,
            }
        )
    res1 = _run(_phase1_nc(), in_maps1, "phase1")

    # hT output layout [128, (m, col)] -> [D, PC] per core
    hT_d = np.concatenate(
        [
            r["hT"].reshape(128, DK, PC).transpose(1, 0, 2).reshape(D, PC)
            for r in res1
        ],
        axis=1,
    )  # [D, NT] f16
    d2 = np.concatenate([r["d2X"] for r in res1], axis=1).T      # [NT, E] f32
    rr = np.concatenate([r["rrO"] for r in res1], axis=1).T      # [NT, 1] f32
    d2 += rr
    d2 += kk[None, :]

    # ---- Host repair: recompute borderline tokens exactly in fp32 ----
    logits0 = -np.sqrt(np.maximum(d2, 0.0), dtype=np.float32)
    part = np.partition(logits0, E - K - 1, axis=1)
    gap45 = part[:, E - K] - part[:, E - K - 1]  # 4th minus 5th logit
    risk = np.nonzero(gap45 < REPAIR_MARGIN)[0]
    last_stats["n_repaired"] = int(risk.size)
    if risk.size:
        x_all = np.concatenate(
            [view0.reshape(N, D), view1.reshape(N, D)], axis=0
        )
        vsel = (risk >= N).astype(np.int64)
        for v in (0, 1):
            rt = risk[vsel == v]
            if rt.size == 0:
                continue
            hx = x_all[rt] @ proj_w[v] + proj_b[v]
            rx = hx @ router_w[v]
            d2[rt] = (
                (rx * rx).sum(axis=1, keepdims=True)
                - 2.0 * (rx @ keys.T)
                + kk
            )

    # ---- Host routing: logits, top-4, softmax gates (fp32) ----
    logits = -np.sqrt(np.maximum(d2, 0.0), dtype=np.float32)
    topi = np.argsort(-logits, axis=1, kind="stable")[:, :K]   # [NT, K]
    topv = np.take_along_axis(logits, topi, axis=1)
    ex = np.exp(topv - topv[:, :1], dtype=np.float32)
    gates = ex / ex.sum(axis=1, keepdims=True, dtype=np.float32)

    # ---- Slot plan ----
    per_e, slots, loads = _plan_slots(topi, gates)
    lens = tuple(sl[0] for sl in slots)
    S = len(lens)
    offs = np.concatenate([[0], np.cumsum(lens)]).astype(np.int64)
    Ctot = int(offs[-1])

    # ---- Phase 2 inputs (partition-major packed) ----
    w1r = np.stack([_pack_pmajor(w1[e].astype(np.float16), DK) for e in range(E)])
    w2r = np.stack([_pack_pmajor(w2[e].astype(np.float16), HK) for e in range(E)])
    hT_p = np.ascontiguousarray(
        hT_d.reshape(DK, 128, NT).transpose(1, 0, 2)
    )  # [128, DK, NT]
    in_maps2 = []
    core_cells = []  # per core: list over slots of (e, toks, gvals) or None
    for c in range(NC):
        hseg = np.zeros((128, DK * Ctot), np.float16)
        w1c = np.zeros((S, 128, DK * H), np.float16)
        w2c = np.zeros((S, 128, HK * D), np.float16)
        b1c = np.zeros((128, S * HK), np.float32)
        b2c = np.zeros((128, S * DK), np.float32)
        cells = []
        for s, (Lc, cell8) in enumerate(slots):
            cell = cell8[c]
            if cell is None:
                cells.append(None)
                continue
            e, start, n = cell
            toks = per_e[e][0][start : start + n]
            gv = per_e[e][1][start : start + n]
            cells.append((e, toks, gv))
            blk = hT_p[:, :, toks]  # [128, DK, n]
            o0 = int(offs[s]) * DK
            hs = hseg[:, o0 : o0 + DK * Lc].reshape(128, DK, Lc)  # strided view
            hs[:, :, :n] = blk
            w1c[s] = w1r[e]
            w2c[s] = w2r[e]
            b1c[:, s * HK : (s + 1) * HK] = b1[e].reshape(HK, 128).T
            b2c[:, s * DK : (s + 1) * DK] = b2[e].reshape(DK, 128).T
        core_cells.append(cells)
        in_maps2.append(
            {"hseg": hseg, "w1s": w1c, "w2s": w2c, "b1s": b1c, "b2s": b2c}
        )
    last_stats["S"] = S
    last_stats["n_slots_real"] = sum(
        1 for cells in core_cells for cl in cells if cl is not None
    )
    last_stats["cols_per_core"] = Ctot
    last_stats["n_wloads"] = sum(loads)
    res2 = _run(_phase2_nc(lens, tuple(loads)), in_maps2, "phase2")

    # ---- Combine (gates applied here) ----
    fusedT = np.zeros((D, NT), np.float32)
    for c in range(NC):
        o = res2[c]["oseg"]  # [128, DK*Ctot]
        for s in range(S):
            cell = core_cells[c][s]
            if cell is None:
                continue
            e, toks, gv = cell
            n = toks.size
            Lc = lens[s]
            o0 = int(offs[s]) * DK
            blk = o[:, o0 : o0 + DK * Lc].reshape(128, DK, Lc)[:, :, :n]
            fusedT[:, toks] += (
                blk.transpose(1, 0, 2).reshape(D, n) * gv[None, :]
            )
    fused = (fusedT[:, :N] + fusedT[:, N:]).T  # [N, D]
    return np.ascontiguousarray(fused.reshape(B, T, D), dtype=np.float32)


# revision 40
# speedup vs baseline: 1.1971x; 1.0018x over previous
"""Trainium2 Bass kernel for nn_MoEElementFusion (moe_routing).

Strategy (8 NeuronCores, SPMD, two launches with host routing in between):
  Phase 1 (token-data-parallel): each core takes 1/8 of the 8192 (view,token)
  columns, computes in fp16 on the PE (psum fp32):
      h   = x @ proj_w + proj_b
      r   = h @ router_w
      d2X = (-2 keys) . r        and   rr = |r|^2
  Host adds |k|^2 + rr, repairs borderline top-4 rows in fp32, takes stable
  top-4 and softmax gates.

  Slot plan: per expert, selected columns cut into full-512 pieces plus a
  remainder; a small search promotes the largest remainders into full cells
  and groups the rest into short tail slots, minimizing the per-core column
  total (SPMD: every core runs the same compile-time slot-length list; which
  expert fills each slot is pure input data).

  Phase 2 (compiled at runtime once the length list is known): per slot,
  FFN in fp16 (1 cycle/row on the PE):
      out^T = w2^T-mm(gelu(w1^T-mm(h^T) + b1)) + b2

  Perf notes (from NTFF traces):
  - Every large DRAM tensor is host-repacked to partition-major layout so
    DMA descriptor lines are 4-16KB (HWDGE queues are descriptor-rate
    limited at small lines).
  - Weight loads are split across both HWDGE trigger queues (sync=SP,
    scalar=ACT), w2 in two chunks per half so its first m-tiles land early;
    output writes and small/latency-insensitive loads ride gpsimd SWDGE.
  - The PE is pre-warmed with a memset-fed matmul chain into a dedicated
    PSUM bank (drain traced at the end of the program so no engine queue
    head-of-line blocks on it) - the HAM clock gate releases (1.2 -> 2.4
    GHz) while the first inputs stream in.
  - All PSUM output drains run on the otherwise-idle DVE so the ACT engine's
    gelu chain is never interrupted at slot boundaries.
  - Gates are applied on the host during the combine.
"""

import math
import os

import numpy as np

import concourse.bass as bass
import concourse.bacc as bacc
import concourse.mybir as mybir
import concourse.tile as tile
from concourse.bass_utils import run_bass_kernel_spmd

# Problem dims (hardcoded per spec)
V, B, T, D, E, K = 2, 4, 1024, 512, 16, 4
H = 4 * D
N = B * T          # tokens per view
NT = V * N         # total (view, token) columns = 8192
NC = 8             # cores
PC = NT // NC      # phase-1 columns per core = 1024

F16 = mybir.dt.float16
F32 = mybir.dt.float32
AF = mybir.ActivationFunctionType
ALU = mybir.AluOpType

DK = D // 128      # 4 k-tiles over D
HK = H // 128      # 16 k-tiles over H

REPAIR_MARGIN = 0.02
NWARM1 = 17        # phase-1 warmup matmuls
NWARM2 = 22        # phase-2 warmup matmuls
MIN_SLOT = 192     # shorter slots would bottleneck on ACT-engine work

# Filled by kernel() for test harness introspection.
last_stats: dict = {}


def _warmup_start(nc, cpool, wpool, nwarm, gelu_warm=False):
    """Matmul chain on a memset tile accumulating into a dedicated PSUM bank.
    No input-DMA dependency: keeps the PE busy from ~4us so the HAM clock
    gate releases while the first inputs stream in."""
    wz_sb = cpool.tile([128, 512], F16, tag="wz")
    nc.vector.memset(wz_sb[:], 0.0)
    wo_sb = cpool.tile([128, 512], F32, tag="wo")
    if gelu_warm:
        # tiny gelu NOW so ACT's 1.3us gelu-table load happens before its
        # queue fills with blocking weight-prefetch triggers
        nc.scalar.activation(wo_sb[:, 0:16], wz_sb[:, 0:16], AF.Gelu)
    wps = wpool.tile([128, 512], F32, tag="wps")
    for i in range(nwarm):
        nc.tensor.matmul(
            wps[:], wz_sb[:, 0:128], wz_sb[:],
            start=(i == 0), stop=(i == nwarm - 1),
        )
    return wps, wo_sb


def _warmup_end(nc, wo_sb, wps, wout):
    """Drain the warmup PSUM bank (DVE) and write the dummy output (gpsimd),
    traced at the END of the program so no engine queue blocks on it."""
    nc.vector.tensor_scalar(wo_sb[:, 16:], wps[:, 16:], 0.0, None, ALU.add)
    nc.gpsimd.dma_start(wout[:], wo_sb[:])


def _phase1_nc() -> bass.Bass:
    nc = bacc.Bacc("TRN2", target_bir_lowering=False, num_devices=NC)
    # Partition-major packed inputs (see host-side packing in kernel()).
    xT = nc.dram_tensor("xT", [128, DK * PC], F16, kind="ExternalInput")
    pw = nc.dram_tensor("pw", [128, DK * D], F16, kind="ExternalInput")
    pb = nc.dram_tensor("pb", [128, DK], F32, kind="ExternalInput")
    rw = nc.dram_tensor("rw", [128, DK * D], F16, kind="ExternalInput")
    kT2 = nc.dram_tensor("kT2", [128, DK * E], F16, kind="ExternalInput")
    onc = nc.dram_tensor("onc", [128, 1], F16, kind="ExternalInput")
    hT = nc.dram_tensor("hT", [128, DK * PC], F16, kind="ExternalOutput")
    d2X = nc.dram_tensor("d2X", [E, PC], F32, kind="ExternalOutput")
    rrO = nc.dram_tensor("rrO", [1, PC], F32, kind="ExternalOutput")
    wout = nc.dram_tensor("wout", [128, 512], F32, kind="ExternalOutput")

    NCH = PC // 512  # 512-column compute chunks

    with tile.TileContext(nc) as tc:
        with (
            tc.tile_pool(name="const", bufs=1) as cpool,
            tc.tile_pool(name="act", bufs=1) as apool,
            tc.tile_pool(name="ps", bufs=2, space="PSUM") as pspool,
            tc.tile_pool(name="ps_small", bufs=2, space="PSUM") as psmall,
            tc.tile_pool(name="ps_warm", bufs=1, space="PSUM") as wpool,
        ):
            wps, wo_sb = _warmup_start(nc, cpool, wpool, NWARM1)

            # Inputs: xT per (512-col chunk, k-half), weights per k-half,
            # interleaved across the two HWDGE trigger queues so the first
            # proj chunk's operands land as early as possible.
            # chunk-major xT tile: each chunk DMA is 4KB-contiguous per
            # partition on both sides (128 descriptors, not 512)
            xT_sb = cpool.tile([128, NCH, DK, 512], F16, tag="xT")
            pw_sb = cpool.tile([128, DK, D], F16, tag="pw")
            rw_sb = cpool.tile([128, DK, D], F16, tag="rw")
            nc.sync.dma_start(xT_sb[:, 0], xT[:, 0 : DK * 512])
            nc.scalar.dma_start(pw_sb[:], pw[:])
            nc.sync.dma_start(xT_sb[:, 1], xT[:, DK * 512 : 2 * DK * 512])
            nc.sync.dma_start(rw_sb[:], rw[:])
            pb_sb = cpool.tile([128, DK], F32, tag="pb")
            nc.gpsimd.dma_start(pb_sb[:], pb[:])
            kT2_sb = cpool.tile([128, DK, E], F16, tag="kT2")
            nc.gpsimd.dma_start(kT2_sb[:], kT2[:])
            onc_sb = cpool.tile([128, 1], F16, tag="onc")
            nc.gpsimd.dma_start(onc_sb[:], onc[:])

            hT_sb = apool.tile([128, DK, PC], F16, tag="hT")
            rT_sb = apool.tile([128, DK, PC], F16, tag="rT")
            r2_sb = apool.tile([128, DK, PC], F16, tag="r2")
            rr_sb = apool.tile([1, PC], F32, tag="rr")
            d2_sb = apool.tile([E, PC], F32, tag="d2")

            # h^T = pw^T-mm(x^T) + pb ; r^T = rw^T-mm(h^T)
            for w_sb, chunk_src, dst, bias in (
                (pw_sb, True, hT_sb, pb_sb),
                (rw_sb, False, rT_sb, None),
            ):
                for m in range(DK):
                    for n in range(NCH):
                        ps = pspool.tile([128, 512], F32, tag="ps")
                        for k in range(DK):
                            rhs = (
                                xT_sb[:, n, k, :]
                                if chunk_src
                                else hT_sb[:, k, n * 512 : (n + 1) * 512]
                            )
                            nc.tensor.matmul(
                                ps[:],
                                w_sb[:, k, m * 128 : (m + 1) * 128],
                                rhs,
                                start=(k == 0),
                                stop=(k == DK - 1),
                            )
                        if bias is not None:
                            nc.scalar.activation(
                                dst[:, m, n * 512 : (n + 1) * 512], ps[:],
                                AF.Identity, bias=bias[:, m : m + 1],
                            )
                        else:
                            nc.scalar.activation(
                                dst[:, m, n * 512 : (n + 1) * 512], ps[:], AF.Copy
                            )
                    if bias is not None:
                        # one whole-row store per m: 2KB lines, 128 descriptors
                        eng = nc.sync if m % 2 else nc.scalar
                        eng.dma_start(
                            hT[:, m * PC : (m + 1) * PC], dst[:, m, :]
                        )

            # rr = sum_d r^2 (ones-matmul over partition tiles)
            for kt in range(DK):
                nc.vector.tensor_mul(
                    r2_sb[:, kt, :], rT_sb[:, kt, :], rT_sb[:, kt, :]
                )
            for n in range(NCH):
                ps1 = psmall.tile([1, 512], F32, tag="ps1")
                for k in range(DK):
                    nc.tensor.matmul(
                        ps1[:], onc_sb[:], r2_sb[:, k, n * 512 : (n + 1) * 512],
                        start=(k == 0), stop=(k == DK - 1),
                    )
                nc.scalar.activation(rr_sb[:, n * 512 : (n + 1) * 512], ps1[:], AF.Copy)
            nc.scalar.dma_start(rrO[:], rr_sb[:])

            # d2X = (-2 keys) . r   (|k|^2 and rr are added on the host)
            for n in range(NCH):
                psA = psmall.tile([E, 512], F32, tag="psA")
                for k in range(DK):
                    nc.tensor.matmul(
                        psA[:], kT2_sb[:, k, :], rT_sb[:, k, n * 512 : (n + 1) * 512],
                        start=(k == 0), stop=(k == DK - 1),
                    )
                nc.scalar.activation(d2_sb[:, n * 512 : (n + 1) * 512], psA[:], AF.Copy)
            nc.sync.dma_start(d2X[:], d2_sb[:])

            _warmup_end(nc, wo_sb, wps, wout)
    nc.compile()
    return nc


def _phase2_nc(lens: tuple, loads: tuple) -> bass.Bass:
    S = len(lens)
    offs = [0]
    for L in lens:
        offs.append(offs[-1] + L)
    Ctot = offs[-1]

    nc = bacc.Bacc("TRN2", target_bir_lowering=False, num_devices=NC)
    # Partition-major packed layouts (16KB DMA lines for weights).
    hseg = nc.dram_tensor("hseg", [128, DK * Ctot], F16, kind="ExternalInput")
    w1s = nc.dram_tensor("w1s", [S, 128, DK * H], F16, kind="ExternalInput")
    w2s = nc.dram_tensor("w2s", [S, 128, HK * D], F16, kind="ExternalInput")
    b1s = nc.dram_tensor("b1s", [128, S * HK], F32, kind="ExternalInput")
    b2s = nc.dram_tensor("b2s", [128, S * DK], F32, kind="ExternalInput")
    oseg = nc.dram_tensor("oseg", [128, DK * Ctot], F32, kind="ExternalOutput")
    wout = nc.dram_tensor("wout", [128, 512], F32, kind="ExternalOutput")

    with tile.TileContext(nc) as tc:
        with (
            tc.tile_pool(name="const", bufs=1) as cpool,
            tc.tile_pool(name="w1p", bufs=3) as w1p,
            tc.tile_pool(name="w2p", bufs=3) as w2p,
            tc.tile_pool(name="hp", bufs=3) as hp,
            tc.tile_pool(name="hidp", bufs=4) as hidp,
            tc.tile_pool(name="op", bufs=6) as op,
            tc.tile_pool(name="hid_ps", bufs=3, space="PSUM") as hidps,
            tc.tile_pool(name="out_ps", bufs=1, space="PSUM") as outps,
            tc.tile_pool(name="ps_warm", bufs=1, space="PSUM") as wpool,
        ):
            wps, wo_sb = _warmup_start(nc, cpool, wpool, NWARM2, gelu_warm=True)

            b1_sb = cpool.tile([128, S * HK], F32, tag="b1")
            nc.gpsimd.dma_start(b1_sb[:], b1s[:])
            b2_sb = cpool.tile([128, S * DK], F32, tag="b2")
            nc.gpsimd.dma_start(b2_sb[:], b2s[:])

            w1t = w2t = None
            for s in range(S):
                Lc = lens[s]
                off = offs[s]
                ht = hp.tile([128, DK, 512], F16, tag="h")
                nc.sync.dma_start(
                    ht[:, :, :Lc], hseg[:, off * DK : off * DK + DK * Lc]
                )
                if loads[s]:
                    w1t = w1p.tile([128, DK, H], F16, tag="w1")
                    w2t = w2p.tile([128, HK, D], F16, tag="w2")
                # else: reuse previous slot's w1t/w2t (same expert per core)
                if not loads[s]:
                    pass
                elif s == 0:
                    # Head-latency path: w1 per (k, m-half) so the first
                    # m-tiles land ~3us earlier; queues are empty here so the
                    # finer descriptor lines don't throttle anything.
                    HH = H // 2
                    for hh in range(2):
                        for k, eng in ((0, nc.sync), (1, nc.sync),
                                       (2, nc.scalar), (3, nc.scalar)):
                            eng.dma_start(
                                w1t[:, k, hh * HH : (hh + 1) * HH],
                                w1s[s, :, k * H + hh * HH : k * H + (hh + 1) * HH],
                            )
                        for q, eng in ((2 * hh, nc.sync), (2 * hh + 1, nc.scalar)):
                            eng.dma_start(
                                w2t[:, 4 * q : 4 * q + 4, :],
                                w2s[s, :, 4 * q * D : (4 * q + 4) * D],
                            )
                else:
                    # Steady state: k-half / m-half loads with 8KB lines keep
                    # the HWDGE descriptor rate low.
                    nc.sync.dma_start(w1t[:, 0:2, :], w1s[s, :, : 2 * H])
                    nc.scalar.dma_start(w1t[:, 2:4, :], w1s[s, :, 2 * H :])
                    nc.sync.dma_start(w2t[:, 0:8, :], w2s[s, :, : 8 * D])
                    nc.scalar.dma_start(w2t[:, 8:16, :], w2s[s, :, 8 * D :])
                opsums = [
                    outps.tile([128, 512], F32, tag=f"opsum{mo}",
                               name=f"opsum_{s}_{mo}")
                    for mo in range(DK)
                ]
                for m in range(HK):
                    hps = hidps.tile([128, 512], F32, tag="hps")
                    for k in range(DK):
                        nc.tensor.matmul(
                            hps[:, :Lc],
                            w1t[:, k, m * 128 : (m + 1) * 128],
                            ht[:, k, :Lc],
                            start=(k == 0),
                            stop=(k == DK - 1),
                        )
                    hidt = hidp.tile([128, 512], F16, tag="hid")
                    nc.scalar.activation(
                        hidt[:, :Lc], hps[:, :Lc], AF.Gelu,
                        bias=b1_sb[:, s * HK + m : s * HK + m + 1],
                    )
                    for mo in range(DK):
                        nc.tensor.matmul(
                            opsums[mo][:, :Lc],
                            w2t[:, m, mo * 128 : (mo + 1) * 128],
                            hidt[:, :Lc],
                            start=(m == 0),
                            stop=(m == HK - 1),
                        )
                # All drains on DVE; separate single-bank opsum tiles release
                # one by one so the next slot's w2 accumulation starts early.
                for mo in range(DK):
                    ot = op.tile([128, 512], F32, tag="o")
                    bcol = b2_sb[:, s * DK + mo : s * DK + mo + 1]
                    nc.vector.tensor_scalar(
                        ot[:, :Lc], opsums[mo][:, :Lc], bcol, None, ALU.add
                    )
                    if s == S - 1:  # final slot: HWDGE only (snappier tail)
                        eng = (nc.sync, nc.scalar, nc.sync, nc.scalar)[mo]
                    else:
                        eng = (nc.gpsimd, nc.gpsimd, nc.sync, nc.scalar)[mo]
                    eng.dma_start(
                        oseg[:, off * DK + mo * Lc : off * DK + (mo + 1) * Lc],
                        ot[:, :Lc],
                    )

            _warmup_end(nc, wo_sb, wps, wout)
    nc.compile()
    return nc


def _run(nc, in_maps, label):
    trace = os.environ.get("KTRACE") == "1"
    res = run_bass_kernel_spmd(
        nc, in_maps, core_ids=list(range(NC)), trace=trace
    )
    if trace:
        last_stats[label] = {
            "exec_time_ns": res.exec_time_ns,
            "mean_exec_time_ns": res.mean_exec_time_ns,
            "trace": res.instructions_and_trace[1]
            if res.instructions_and_trace
            else None,
        }
    return res.results


def _round16(x: int) -> int:
    return (x + 7) & ~7


def _pack_pmajor(a: np.ndarray, kt: int) -> np.ndarray:
    """[kt*128, F] row-major -> [128, kt*F] partition-major."""
    f = a.shape[1]
    return np.ascontiguousarray(
        a.reshape(kt, 128, f).transpose(1, 0, 2).reshape(128, kt * f)
    )


def _plan_slots(topi, gates):
    """Cut each expert's selected columns into pieces and pick compile-time
    slot lengths minimizing the per-core column total (SPMD-uniform)."""
    per_e = []  # (token_ids, gate_vals)
    for e in range(E):
        sel_tok, sel_k = np.nonzero(topi == e)
        per_e.append((sel_tok, gates[sel_tok, sel_k]))

    full_pieces = []  # (e, start, 512)
    rem_pieces = []   # (e, start, len)
    for e in range(E):
        n = per_e[e][0].size
        f, r = divmod(n, 512)
        for i in range(f):
            full_pieces.append((e, i * 512, 512))
        if r:
            rem_pieces.append((e, f * 512, r))
    rem_pieces.sort(key=lambda p: -p[2])

    Fn = len(full_pieces)
    best = None
    for p in range(len(rem_pieces) + 1):
        n512 = Fn + p
        S512 = max(1, math.ceil(n512 / NC))
        rest = rem_pieces[p:]
        tlens = [
            max(MIN_SLOT, _round16(rest[i * NC][2]))
            for i in range(math.ceil(len(rest) / NC))
        ]
        cost = 512 * S512 + sum(tlens)
        if best is None or cost < best[0]:
            best = (cost, p, S512, tlens)
    _, p, S512, tlens = best

    full_cells = full_pieces + rem_pieces[:p]
    full_cells += [None] * (S512 * NC - len(full_cells))
    rest = rem_pieces[p:]

    # Same-expert cell PAIRS per core let the second slot of a pair skip its
    # 4MiB weight load (compile-time flag, SPMD-uniform): group full cells by
    # expert, emit floor(pairs/8) slot-pairs, the rest as unpaired slots.
    by_e = {}
    for cell in full_cells:
        if cell is not None:
            by_e.setdefault(cell[0], []).append(cell)
    pairs = []
    singles = []
    for e, cells in by_e.items():
        n2 = len(cells) // 2
        for i in range(n2):
            pairs.append((cells[2 * i], cells[2 * i + 1]))
        if len(cells) % 2:
            singles.append(cells[-1])
    n_sp = len(pairs) // NC
    slots = []   # (length, [8 cells])
    loads = []   # True if slot s loads fresh weights
    for j in range(n_sp):
        grp = pairs[j * NC : (j + 1) * NC]
        slots.append((512, [g[0] for g in grp]))
        loads.append(True)
        slots.append((512, [g[1] for g in grp]))
        loads.append(False)
    for pr in pairs[n_sp * NC :]:
        singles.extend(pr)
    singles += [None] * ((-len(singles)) % NC)
    for i in range(len(singles) // NC):
        slots.append((512, singles[i * NC : (i + 1) * NC]))
        loads.append(True)
    for i, tl in enumerate(tlens):
        cells = rest[i * NC : (i + 1) * NC]
        cells += [None] * (NC - len(cells))
        slots.append((tl, cells))  # short slots last: smaller kernel tail
        loads.append(True)
    return per_e, slots, loads


def kernel(view0, view1, proj_w, proj_b, router_w, expert_keys, w1, b1, w2, b2):
    view0 = np.ascontiguousarray(view0, dtype=np.float32)
    view1 = np.ascontiguousarray(view1, dtype=np.float32)
    proj_w = np.asarray(proj_w, dtype=np.float32)
    proj_b = np.asarray(proj_b, dtype=np.float32)
    router_w = np.asarray(router_w, dtype=np.float32)
    keys = np.asarray(expert_keys, dtype=np.float32)
    w1 = np.asarray(w1, dtype=np.float32)
    b1 = np.asarray(b1, dtype=np.float32)
    w2 = np.asarray(w2, dtype=np.float32)
    b2 = np.asarray(b2, dtype=np.float32)

    # ---- Phase 1: h, cross-term d2X and rr on device ----
    xT_full = np.concatenate(
        [view0.reshape(N, D).T, view1.reshape(N, D).T], axis=1
    ).astype(np.float16)  # [D, NT], column t = view*N + (b*T + tt)

    kT2 = _pack_pmajor(np.ascontiguousarray(-2.0 * keys.T).astype(np.float16), DK)
    kk = (keys * keys).sum(axis=1, dtype=np.float32)  # [E]
    onc = np.ones((128, 1), np.float16)

    in_maps1 = []
    for c in range(NC):
        v = (c * PC) // N  # cores 0-3 -> view 0, 4-7 -> view 1
        xc = xT_full[:, c * PC : (c + 1) * PC]  # [D, PC]
        # chunk-contiguous packing: [128, (n512, k, c)]
        xr = np.ascontiguousarray(
            xc.reshape(DK, 128, PC // 512, 512)
            .transpose(1, 2, 0, 3)
            .reshape(128, DK * PC)
        )
        in_maps1.append(
            {
                "xT": xr,
                "pw": _pack_pmajor(proj_w[v].astype(np.float16), DK),
                "pb": np.ascontiguousarray(proj_b[v].reshape(DK, 128).T),
                "rw": _pack_pmajor(router_w[v].astype(np.float16), DK),
                "kT2": kT2,
                "onc": onc,
            }
        )
    res1 = _run(_phase1_nc(), in_maps1, "phase1")

    # hT output layout [128, (m, col)] -> [D, PC] per core
    hT_d = np.concatenate(
        [
            r["hT"].reshape(128, DK, PC).transpose(1, 0, 2).reshape(D, PC)
            for r in res1
        ],
        axis=1,
    )  # [D, NT] f16
    d2 = np.concatenate([r["d2X"] for r in res1], axis=1).T      # [NT, E] f32
    rr = np.concatenate([r["rrO"] for r in res1], axis=1).T      # [NT, 1] f32
    d2 += rr
    d2 += kk[None, :]

    # ---- Host repair: recompute borderline tokens exactly in fp32 ----
    logits0 = -np.sqrt(np.maximum(d2, 0.0), dtype=np.float32)
    part = np.partition(logits0, E - K - 1, axis=1)
    gap45 = part[:, E - K] - part[:, E - K - 1]  # 4th minus 5th logit
    risk = np.nonzero(gap45 < REPAIR_MARGIN)[0]
    last_stats["n_repaired"] = int(risk.size)
    if risk.size:
        x_all = np.concatenate(
            [view0.reshape(N, D), view1.reshape(N, D)], axis=0
        )
        vsel = (risk >= N).astype(np.int64)
        for v in (0, 1):
            rt = risk[vsel == v]
            if rt.size == 0:
                continue
            hx = x_all[rt] @ proj_w[v] + proj_b[v]
            rx = hx @ router_w[v]
            d2[rt] = (
                (rx * rx).sum(axis=1, keepdims=True)
                - 2.0 * (rx @ keys.T)
                + kk
            )

    # ---- Host routing: logits, top-4, softmax gates (fp32) ----
    logits = -np.sqrt(np.maximum(d2, 0.0), dtype=np.float32)
    topi = np.argsort(-logits, axis=1, kind="stable")[:, :K]   # [NT, K]
    topv = np.take_along_axis(logits, topi, axis=1)
    ex = np.exp(topv - topv[:, :1], dtype=np.float32)
    gates = ex / ex.sum(axis=1, keepdims=True, dtype=np.float32)

    # ---- Slot plan ----
    per_e, slots, loads = _plan_slots(topi, gates)
    lens = tuple(sl[0] for sl in slots)
    S = len(lens)
    offs = np.concatenate([[0], np.cumsum(lens)]).astype(np.int64)
    Ctot = int(offs[-1])

    # ---- Phase 2 inputs (partition-major packed) ----
    w1r = np.stack([_pack_pmajor(w1[e].astype(np.float16), DK) for e in range(E)])
    w2r = np.stack([_pack_pmajor(w2[e].astype(np.float16), HK) for e in range(E)])
    hT_p = np.ascontiguousarray(
        hT_d.reshape(DK, 128, NT).transpose(1, 0, 2)
    )  # [128, DK, NT]
    in_maps2 = []
    core_cells = []  # per core: list over slots of (e, toks, gvals) or None
    for c in range(NC):
        hseg = np.zeros((128, DK * Ctot), np.float16)
        w1c = np.zeros((S, 128, DK * H), np.float16)
        w2c = np.zeros((S, 128, HK * D), np.float16)
        b1c = np.zeros((128, S * HK), np.float32)
        b2c = np.zeros((128, S * DK), np.float32)
        cells = []
        for s, (Lc, cell8) in enumerate(slots):
            cell = cell8[c]
            if cell is None:
                cells.append(None)
                continue
            e, start, n = cell
            toks = per_e[e][0][start : start + n]
            gv = per_e[e][1][start : start + n]
            cells.append((e, toks, gv))
            blk = hT_p[:, :, toks]  # [128, DK, n]
            o0 = int(offs[s]) * DK
            hs = hseg[:, o0 : o0 + DK * Lc].reshape(128, DK, Lc)  # strided view
            hs[:, :, :n] = blk
            w1c[s] = w1r[e]
            w2c[s] = w2r[e]
            b1c[:, s * HK : (s + 1) * HK] = b1[e].reshape(HK, 128).T
            b2c[:, s * DK : (s + 1) * DK] = b2[e].reshape(DK, 128).T
        core_cells.append(cells)
        in_maps2.append(
            {"hseg": hseg, "w1s": w1c, "w2s": w2c, "b1s": b1c, "b2s": b2c}
        )
    last_stats["S"] = S
    last_stats["n_slots_real"] = sum(
        1 for cells in core_cells for cl in cells if cl is not None
    )
    last_stats["cols_per_core"] = Ctot
    last_stats["n_wloads"] = sum(loads)
    res2 = _run(_phase2_nc(lens, tuple(loads)), in_maps2, "phase2")

    # ---- Combine (gates applied here) ----
    fusedT = np.zeros((D, NT), np.float32)
    for c in range(NC):
        o = res2[c]["oseg"]  # [128, DK*Ctot]
        for s in range(S):
            cell = core_cells[c][s]
            if cell is None:
                continue
            e, toks, gv = cell
            n = toks.size
            Lc = lens[s]
            o0 = int(offs[s]) * DK
            blk = o[:, o0 : o0 + DK * Lc].reshape(128, DK, Lc)[:, :, :n]
            fusedT[:, toks] += (
                blk.transpose(1, 0, 2).reshape(D, n) * gv[None, :]
            )
    fused = (fusedT[:, :N] + fusedT[:, N:]).T  # [N, D]
    return np.ascontiguousarray(fused.reshape(B, T, D), dtype=np.float32)
